# revision 25
# baseline (speedup 1.0000x reference)
# Trainium2 Bass kernel for the NeuralODE problem (Tsit5, data-parallel over batch).
#
# Sharding: batch 4096 -> 8 cores x 512.  Per core, the 512 trajectories are
# packed 4 batch-groups x 128 on SBUF partitions ("g4" layout): a tensor with
# F features lives as [4*F partitions, 128 free].
#
# Numerics: matmul operands fp16 (PSUM accumulates fp32), integration state z
# and saved trajectory fp32.  All swish betas, layer biases, the dat1 layer,
# the ctx-net second layer, and the Tsit5 stage combinations are folded into
# host-precomputed block-diagonal weights / activation bias vectors, so the
# per-stage device work is: k-combine matmuls -> 1 DVE op -> dat0 matmul ->
# silu -> 3x (main matmul triple + silu).
#
# The context signal is z-independent: ctx_beta*W1@deltas is precomputed on
# device into P1/DD tables (value + forward difference per grid interval); per
# substep the 6 stage contexts are linearly interpolated with baked fp32 fracs
# and passed through one batched silu per chunk.

import numpy as np

B_FULL, T, D = 4096, 64, 2
NCORES = 8
BC = B_FULL // NCORES        # 512 batch per core
G = 4                        # partition packing groups
F = BC // G                  # 128 free elems per group
L, H = 8, 32
CTXD, HID = 3, 49
HA, HB = 32, HID - 32        # ctx hidden split (49 = 32 + 17)
NSTEP = T - 1
SUBSTEPS = 2
STREAMS = 2

# Tsit5 tableau (matches reference.py)
C2, C3, C4, C5 = 0.161, 0.327, 0.9, 0.9800255409045097
A = [
    [],
    [0.161],
    [-0.008480655492356989, 0.335480655492357],
    [2.8971530571054935, -6.359448489975075, 4.3622954328695815],
    [5.325864828439257, -11.748883564062828, 7.4955393428898365, -0.09249506636175525],
    [5.86145544294642, -12.92096931784711, 8.159367898576159, -0.071584973281401,
     -0.028269050394068383],
]
BROW = [0.09646076681806523, 0.01, 0.4798896504144996, 1.379008574103742,
        -3.290069515436081, 2.324710524099774]
CS = [0.0, C2, C3, C4, C5, 1.0]

F16 = np.float16
F32 = np.float32


def _bd(W, g=G):
    """W [out, in] -> block-diagonal lhsT [g*in, g*out] (group-major)."""
    W = np.asarray(W, np.float64)
    out_d, in_d = W.shape
    M = np.zeros((g * in_d, g * out_d), dtype=np.float64)
    for k in range(g):
        M[k * in_d:(k + 1) * in_d, k * out_d:(k + 1) * out_d] = W.T
    return M


def _rep(v, g=G):
    """bias vector [F] -> replicated per-partition column [g*F, 1]."""
    return np.tile(np.asarray(v, np.float64).reshape(-1), g).reshape(-1, 1)


def _prep_host(xs, t_eval, params):
    """Host-side preprocessing: folded weights, interp schedule, input relayout."""
    xs = np.asarray(xs, F32)
    t_eval = np.asarray(t_eval, F32)
    enc = [(np.asarray(w, F32), np.asarray(b, F32)) for w, b in params["enc"]]
    W1, b1 = [np.asarray(a, F32) for a in params["ctx"][0]]
    W2, b2 = [np.asarray(a, F32) for a in params["ctx"][1]]
    bc_ = float(np.asarray(params["ctx_beta"][0]).reshape(()))
    W0d, b0d = [np.asarray(a, F32) for a in params["dat"][0]]
    W1d, b1d = [np.asarray(a, F32) for a in params["dat"][1]]
    bd_ = float(np.asarray(params["dat_beta"][0]).reshape(()))
    mains = [(np.asarray(w, F32), np.asarray(b, F32)) for w, b in params["main"]]
    betas = [float(np.asarray(b).reshape(())) for b in params["main_beta"]]
    Wk, bk = mains[3]
    Wd, bdec = [np.asarray(a, F32) for a in params["dec"]]

    t0 = F32(t_eval[0])
    dt = F32(t_eval[1] - t_eval[0])
    h = F32(dt / SUBSTEPS)
    hf = float(h)

    consts = {}

    # ---- encoder (fp32, block-diagonal) ----
    for i, (w, b) in enumerate(enc[:-1]):
        consts[f"encw{i}"] = _bd(w).astype(F32)
        consts[f"encb{i}"] = _rep(b).astype(F32)
    # last layer: permute outputs so mu sits at partitions [0,32) (g-major)
    # and logvar at [32,64) — partition slices must start at multiples of 32.
    w3, b3 = enc[-1]
    M3 = np.zeros((G * H, G * 2 * L))
    b3v = np.zeros((G * 2 * L, 1))
    for g in range(G):
        for f in range(2 * L):
            col = g * L + f if f < L else G * L + g * L + (f - L)
            M3[g * H:(g + 1) * H, col] = w3[f]
            b3v[col, 0] = b3[f]
    consts["encw3"] = M3.astype(F32)
    consts["encb3"] = b3v.astype(F32)

    # ---- ctx layer 1: weights for building P1 tables from deltas ----
    # P1 = bc*(W1 @ deltas); deltas split into x-channels and t-channel.
    W1s = bc_ * W1.astype(np.float64)                      # [49, 3]
    chunks = {"a": W1s[:HA], "b": W1s[HA:]}
    for nm, Wc in chunks.items():
        hh = Wc.shape[0]
        Mx = np.zeros((G * 2, G * hh))
        Mt = np.zeros((G, G * hh))
        for g in range(G):
            Mx[g * 2:(g + 1) * 2, g * hh:(g + 1) * hh] = Wc[:, :2].T
            Mt[g, g * hh:(g + 1) * hh] = Wc[:, 2]
        consts[f"p1{nm}x"] = Mx.astype(F16)
        consts[f"p1{nm}t"] = Mt.astype(F16)
    consts["ctxba"] = _rep(bc_ * b1[:HA]).astype(F32)      # silu bias, a-chunk
    consts["ctxbb"] = _rep(bc_ * b1[HA:]).astype(F32)

    # ---- merged main weights ----
    W2s = W2.astype(np.float64) / bc_                      # undo c1 beta scale
    Wys = [m[0][:, :H].astype(np.float64) for m in mains[:3]]
    Wcs = [m[0][:, H:].astype(np.float64) for m in mains[:3]]
    bms = [m[1].astype(np.float64) for m in mains[:3]]
    b1d_ = b1d.astype(np.float64)
    b2_ = b2.astype(np.float64)

    consts["my0"] = _bd(betas[0] * (Wys[0] @ W1d.astype(np.float64)) / bd_).astype(F16)
    consts["my1"] = _bd(betas[1] * Wys[1] / betas[0]).astype(F16)
    consts["my2"] = _bd(betas[2] * Wys[2] / betas[1]).astype(F16)
    for m in range(3):
        Mc = betas[m] * (Wcs[m] @ W2s[H * m:H * (m + 1), :])   # [32, 49]
        consts[f"mca{m}"] = _bd(Mc[:, :HA]).astype(F16)        # [128, 128]
        consts[f"mcb{m}"] = _bd(Mc[:, HA:]).astype(F16)        # [68, 128]
    consts["mb0"] = _rep(betas[0] * (bms[0] + Wys[0] @ b1d_
                                     + Wcs[0] @ b2_[0:H])).astype(F32)
    consts["mb1"] = _rep(betas[1] * (bms[1] + Wcs[1] @ b2_[H:2 * H])).astype(F32)
    consts["mb2"] = _rep(betas[2] * (bms[2] + Wcs[2] @ b2_[2 * H:])).astype(F32)

    # ---- dat0 ----
    consts["w0d"] = _bd(bd_ * W0d).astype(F16)             # [32, 128]
    W0bk = bd_ * (W0d.astype(np.float64) @ bk.astype(np.float64))  # [32]
    delta_s = [hf * sum(row) for row in A]                 # 0 for stage 1
    for e in range(6):
        consts[f"d0b{e}"] = _rep(bd_ * b0d.astype(np.float64)
                                 + delta_s[e] * W0bk).astype(F32)

    # ---- k-combination weight: h * (Wk / beta2); stage coefficients are
    # applied on DVE/Pool (U-combine), exploiting linearity of the matmul.
    Wks = Wk.astype(np.float64) / betas[2]                 # [8, 32]
    consts["wkp"] = _bd(hf * Wks).astype(F16)              # [128, 32]
    consts["bkB"] = _rep(hf * sum(BROW) * bk.astype(np.float64)).astype(F32)

    # ---- decoder ----
    consts["wdec"] = _bd(Wd).astype(F16)                   # [32, 8]
    consts["bdec"] = _rep(bdec).astype(F32)                # [8, 1]

    # ---- interp schedule: (i, frac) per (n, s, stage), fp32-faithful ----
    sched = []
    for n in range(NSTEP):
        for s in range(SUBSTEPS):
            tstart = F32(t_eval[n] + F32(s) * h)
            row = []
            for e in range(6):
                if e == 0:
                    te = tstart
                elif e == 5:
                    te = F32(tstart + h)
                else:
                    te = F32(tstart + F32(F32(CS[e]) * h))
                pos = F32((te - t0) / dt)
                i = int(np.clip(np.floor(pos), 0, T - 2))
                frac = float(F32(pos - F32(i)))
                row.append((i, frac))
            sched.append(((n, s), row))

    # ---- xs relayout: xs (B, T, 2) -> per-core [8 (g*2+d), T*128] ----
    xs_sh = xs.reshape(NCORES, G, F, T, D)                 # [c, g, b, t, d]
    xsg = np.ascontiguousarray(xs_sh.transpose(0, 1, 4, 3, 2)).reshape(
        NCORES, G * D, T * F)

    # ---- pack consts into one fp16 and one fp32 image (single DMA each) ----
    layout = {}
    packed = {}
    for dt_np, img in ((F16, "pk16"), (F32, "pk32")):
        algn = 128 // np.dtype(dt_np).itemsize      # 128B alignment in elems
        off = 0
        entries = []
        for k, v in consts.items():
            if v.dtype != dt_np:
                continue
            p, w = v.shape
            entries.append((k, p, off, w))
            off += ((w + algn - 1) // algn) * algn
        image = np.zeros((128, off), dt_np)
        for k, p, o, w in entries:
            image[:p, o:o + w] = consts[k]
            layout[k] = (img, p, o, w)
        packed[img] = image

    return consts, layout, packed, sched, xsg, float(dt)


_CACHE = {}


def _build(n_substeps, dt_val, layout, packed, sched):
    import concourse.mybir as mybir
    import concourse.tile as tile
    from concourse import bacc
    from concourse.alu_op_type import AluOpType
    from contextlib import ExitStack

    dt16 = mybir.dt.float16
    dt32 = mybir.dt.float32
    AF = mybir.ActivationFunctionType
    ADD, MULT, SUB = AluOpType.add, AluOpType.mult, AluOpType.subtract

    nc = bacc.Bacc(trn_type="TRN2", target_bir_lowering=False, debug=False)

    # ---- dram io ----
    xsg_d = nc.declare_dram_parameter("xsg", [G * D, T * F], dt32, isOutput=False)
    pk_d = {}
    for img, arr in packed.items():
        pk_d[img] = nc.declare_dram_parameter(
            img, list(arr.shape), mybir.dt.from_np(arr.dtype), isOutput=False)
    xr_d = nc.declare_dram_parameter("xr", [G * D, T * F], dt32, isOutput=True)
    mulv_d = nc.declare_dram_parameter("mulv", [G * 2 * L, F], dt32, isOutput=True)
    zlast_d = nc.declare_dram_parameter("zlast", [G * L, F], dt32, isOutput=True)

    with tile.TileContext(nc) as tc, ExitStack() as ctx:
        cpool = ctx.enter_context(tc.tile_pool(name="consts", bufs=1))
        spool = ctx.enter_context(tc.tile_pool(name="state", bufs=1))
        wpool = ctx.enter_context(tc.tile_pool(name="work", bufs=3))
        upool = ctx.enter_context(tc.tile_pool(name="acts", bufs=2))

        pk_t = {}
        for img, arr in packed.items():
            t = cpool.tile(list(arr.shape), mybir.dt.from_np(arr.dtype),
                           tag=f"c_{img}")
            nc.sync.dma_start(out=t, in_=pk_d[img].ap())
            pk_t[img] = t
        ct = {}
        for k, (img, p, o, w) in layout.items():
            ct[k] = pk_t[img][0:p, o:o + w]

        # persistent state
        zs_buf = spool.tile([G * L, T * F], dt32, tag="zs_buf")
        ztmp = spool.tile([G * L, F], dt32, tag="ztmp")
        mulv = spool.tile([G * 2 * L, F], dt32, tag="mulv")
        p1a = spool.tile([G * HA, T * F], dt16, tag="p1a")
        p1b = spool.tile([G * HB, T * F], dt16, tag="p1b")
        dda = spool.tile([G * HA, (T - 1) * F], dt16, tag="dda")
        ddb = spool.tile([G * HB, (T - 1) * F], dt16, tag="ddb")

        # ---------- phase 1: load xs, encoder, deltas, P1/DD ----------
        with tc.tile_pool(name="load", bufs=1) as lpool, \
             tc.tile_pool(name="ppsum", bufs=1, space="PSUM") as ppp:
            xsg = lpool.tile([G * D, T * F], dt32, tag="xsg")
            nc.sync.dma_start(out=xsg, in_=xsg_d.ap())

            # encoder (fp32)
            def softplus(psum_ap, bias_t, out_tile):
                tmp = lpool.tile([128, F], dt32, tag="enc_tmp", bufs=2)
                nc.scalar.activation(tmp, psum_ap, AF.Exp, bias=bias_t)
                nc.vector.tensor_scalar_add(tmp, tmp, 1.0)
                nc.scalar.activation(out_tile, tmp, AF.Ln)

            x0 = xsg[:, 0:F]
            ep = ppp.tile([128, F], dt32, tag="enc_psum", space="PSUM", bufs=2)
            nc.tensor.matmul(ep, lhsT=ct["encw0"], rhs=x0, start=True, stop=True)
            sp = lpool.tile([128, F], dt32, tag="enc_sp0")
            softplus(ep, ct["encb0"], sp)
            for i in (1, 2):
                ep2 = ppp.tile([128, F], dt32, tag="enc_psum", space="PSUM", bufs=2)
                nc.tensor.matmul(ep2, lhsT=ct[f"encw{i}"], rhs=sp,
                                 start=True, stop=True)
                sp2 = lpool.tile([128, F], dt32, tag=f"enc_sp{i}")
                softplus(ep2, ct[f"encb{i}"], sp2)
                sp = sp2
            zp = ppp.tile([G * 2 * L, F], dt32, tag="enc_psum", space="PSUM", bufs=2)
            nc.tensor.matmul(zp, lhsT=ct["encw3"], rhs=sp, start=True, stop=True)
            nc.vector.tensor_scalar_add(mulv, zp, ct["encb3"])
            nc.sync.dma_start(out=mulv_d.ap(), in_=mulv)
            # z0 = mu = rows [0, 32) of mulv (g-major)
            nc.vector.tensor_copy(zs_buf[:, 0:F], mulv[0:G * L, :])

            # deltas
            dx = lpool.tile([G * D, T * F], dt16, tag="dx")
            nc.vector.memset(dx[:, 0:F], 0.0)
            nc.vector.tensor_tensor(dx[:, F:], xsg[:, F:], xsg[:, :(T - 1) * F], SUB)
            dtch = lpool.tile([G, T * F], dt16, tag="dtch")
            nc.vector.memset(dtch[:, 0:F], 0.0)
            nc.vector.memset(dtch[:, F:], dt_val)

            # P1 tables
            for dst, xw, tw in ((p1a, "p1ax", "p1at"), (p1b, "p1bx", "p1bt")):
                for c0 in range(0, T * F, 512):
                    pp = ppp.tile([dst.shape[0], 512], dt32, tag="p1_psum",
                                  space="PSUM", bufs=2)
                    nc.tensor.matmul(pp, lhsT=ct[xw], rhs=dx[:, c0:c0 + 512],
                                     start=True, stop=False)
                    nc.tensor.matmul(pp, lhsT=ct[tw], rhs=dtch[:, c0:c0 + 512],
                                     start=False, stop=True)
                    nc.vector.tensor_copy(dst[:, c0:c0 + 512], pp)
            nc.vector.tensor_tensor(dda, p1a[:, F:], p1a[:, :(T - 1) * F], SUB)
            nc.vector.tensor_tensor(ddb, p1b[:, F:], p1b[:, :(T - 1) * F], SUB)

        # ---------- phase 2: main loop ----------
        # S independent batch streams (columns of F split S ways) pipeline
        # through the engines so the mm -> silu dependency chains overlap.
        S = STREAMS
        FS = F // S
        z16 = []
        for st in range(S):
            csl = slice(st * FS, (st + 1) * FS)
            zt = wpool.tile([G * L, FS], dt16, tag=f"z16_{st}")
            nc.vector.tensor_copy(zt, zs_buf[:, csl])
            z16.append(zt)
        zcur = [zs_buf[:, st * FS:(st + 1) * FS] for st in range(S)]

        def ucombine(eng, terms, utiles, tag):
            """V = sum_j terms[j] * utiles[j] on DVE or Pool, fp16."""
            acc = wpool.tile([128, FS], dt16, tag=tag)
            eng.tensor_scalar_mul(acc, utiles[0], float(terms[0]))
            for j in range(1, len(terms)):
                nxt = wpool.tile([128, FS], dt16, tag=tag)
                eng.scalar_tensor_tensor(nxt, utiles[j], float(terms[j]), acc,
                                         MULT, ADD)
                acc = nxt
            return acc

        with tc.tile_pool(name="mpsum", bufs=1, space="PSUM") as mpp:
            for (n, s), row in sched[:n_substeps]:
                # ctx block: lerp pre-acts for the 6 stages (on GpSimd) and
                # batched silu; all z-independent, runs ahead of the chain.
                prea = wpool.tile([G * HA, 6 * F], dt16, tag="prea")
                preb = wpool.tile([G * HB, 6 * F], dt16, tag="preb")
                for e, (i_, fr) in enumerate(row):
                    sl = slice(i_ * F, (i_ + 1) * F)
                    nc.vector.scalar_tensor_tensor(
                        prea[:, e * F:(e + 1) * F], dda[:, sl], float(fr),
                        p1a[:, sl], MULT, ADD)
                    nc.vector.scalar_tensor_tensor(
                        preb[:, e * F:(e + 1) * F], ddb[:, sl], float(fr),
                        p1b[:, sl], MULT, ADD)
                c1a = wpool.tile([G * HA, 6 * F], dt16, tag="c1a")
                c1b = wpool.tile([G * HB, 6 * F], dt16, tag="c1b")
                nc.scalar.activation(c1a, prea, AF.Silu, bias=ct["ctxba"])
                nc.scalar.activation(c1b, preb, AF.Silu, bias=ct["ctxbb"])

                # 6 Tsit5 stages, S pipelined streams.  The ctx matmuls
                # (mca/mcb) cover both streams in one N=128 mm and are
                # z-independent, so they run ahead; only the y-path matmul
                # and silu sit on each stream's dependency chain.
                us = [[] for _ in range(S)]
                for e in range(6):
                    zrhs = []
                    for st in range(S):
                        if e == 0:
                            zrhs.append(z16[st])
                        else:
                            V = ucombine(nc.vector, A[e], us[st], f"vcomb_{st}")
                            kp = mpp.tile([G * L, FS], dt32, tag=f"kp_{st}",
                                          space="PSUM", bufs=1)
                            nc.tensor.matmul(kp, lhsT=ct["wkp"], rhs=V,
                                             start=True, stop=True)
                            zh = wpool.tile([G * L, FS], dt16, tag=f"zhat_{st}")
                            nc.vector.scalar_tensor_tensor(zh, kp, 1.0, zcur[st],
                                                           MULT, ADD)
                            zrhs.append(zh)
                    ys = []
                    for st in range(S):
                        p0 = mpp.tile([128, FS], dt32, tag=f"mp_{st}",
                                      space="PSUM", bufs=2)
                        nc.tensor.matmul(p0, lhsT=ct["w0d"], rhs=zrhs[st],
                                         start=True, stop=True)
                        u1 = upool.tile([128, FS], dt16, tag=f"u1_{e}_{st}")
                        nc.scalar.activation(u1, p0, AF.Silu, bias=ct[f"d0b{e}"])
                        ys.append(u1)
                    for m in range(3):
                        for st in range(S):
                            pm = mpp.tile([128, FS], dt32, tag=f"mp_{st}",
                                          space="PSUM", bufs=2)
                            esl = slice(e * F + st * FS, e * F + (st + 1) * FS)
                            nc.tensor.matmul(pm, lhsT=ct[f"mca{m}"],
                                             rhs=c1a[:, esl],
                                             start=True, stop=False)
                            nc.tensor.matmul(pm, lhsT=ct[f"mcb{m}"],
                                             rhs=c1b[:, esl],
                                             start=False, stop=False)
                            nc.tensor.matmul(pm, lhsT=ct[f"my{m}"],
                                             rhs=ys[st], start=False, stop=True)
                            ym = upool.tile(
                                [128, FS], dt16,
                                tag=(f"u5_{e}_{st}" if m == 2
                                     else f"y{m}_{e}_{st}"))
                            nc.scalar.activation(ym, pm, AF.Silu,
                                                 bias=ct[f"mb{m}"])
                            ys[st] = ym
                    for st in range(S):
                        us[st].append(ys[st])

                # z update per stream
                for st in range(S):
                    VB = ucombine(nc.vector, BROW, us[st], f"vcomb_{st}")
                    pB = mpp.tile([G * L, FS], dt32, tag=f"kp_{st}",
                                  space="PSUM", bufs=1)
                    nc.tensor.matmul(pB, lhsT=ct["wkp"], rhs=VB,
                                     start=True, stop=True)
                    base = ztmp if s == 0 else zs_buf[:, (n + 1) * F:(n + 2) * F]
                    znew = base[:, st * FS:(st + 1) * FS]
                    nc.vector.scalar_tensor_tensor(znew, pB, ct["bkB"], zcur[st],
                                                   ADD, ADD)
                    zt = wpool.tile([G * L, FS], dt16, tag=f"z16_{st}")
                    nc.vector.tensor_copy(zt, znew)
                    z16[st] = zt
                    zcur[st] = znew

        # ---------- phase 3: decoder ----------
        with tc.tile_pool(name="dpool", bufs=2) as dpool, \
             tc.tile_pool(name="dpsum", bufs=2, space="PSUM") as dpp:
            for c0 in range(0, T * F, 512):
                z16c = dpool.tile([G * L, 512], dt16, tag="dec_z16")
                nc.vector.tensor_copy(z16c, zs_buf[:, c0:c0 + 512])
                dp = dpp.tile([G * D, 512], dt32, tag="dec_psum", space="PSUM")
                nc.tensor.matmul(dp, lhsT=ct["wdec"], rhs=z16c,
                                 start=True, stop=True)
                xrc = dpool.tile([G * D, 512], dt32, tag="dec_out")
                nc.vector.tensor_scalar_add(xrc, dp, ct["bdec"])
                nc.sync.dma_start(out=xr_d.ap()[:, c0:c0 + 512], in_=xrc)
            nc.sync.dma_start(out=zlast_d.ap(), in_=zs_buf[:, (T - 1) * F:])

    nc.compile()
    return nc


def kernel(xs, t_eval, params):
    from concourse.bass_utils import run_bass_kernel_spmd

    consts, layout, packed, sched, xsg, dt_val = _prep_host(xs, t_eval, params)

    if "prog" not in _CACHE:
        _CACHE["prog"] = _build(NSTEP * SUBSTEPS, dt_val, layout, packed, sched)
    nc = _CACHE["prog"]

    in_maps = []
    for c in range(NCORES):
        m = {k: np.ascontiguousarray(v) for k, v in packed.items()}
        m["xsg"] = np.ascontiguousarray(xsg[c])
        in_maps.append(m)

    res = run_bass_kernel_spmd(nc, in_maps, core_ids=list(range(NCORES)))
    outs = res.results

    x_recons = np.empty((B_FULL, T, D), F32)
    z_last = np.empty((B_FULL, L), F32)
    mu = np.empty((B_FULL, L), F32)
    logvar = np.empty((B_FULL, L), F32)
    for c in range(NCORES):
        xr = outs[c]["xr"].reshape(G, D, T, F)
        mlv = outs[c]["mulv"].reshape(2, G, L, F)
        zl = outs[c]["zlast"].reshape(G, L, F)
        for g in range(G):
            sl = slice(c * BC + g * F, c * BC + (g + 1) * F)
            x_recons[sl] = xr[g].transpose(2, 1, 0)
            mu[sl] = mlv[0, g].T
            logvar[sl] = mlv[1, g].T
            z_last[sl] = zl[g].T
    return x_recons, z_last, (mu, logvar)
